# revision 3
# baseline (speedup 1.0000x reference)
"""Head-parallel MultiHeadAttention kernel for 8 Trainium2 NeuronCores.

Problem: B=2, S=2048, D=512, H=8, per-head full-width projections.
Sharding: head h -> core h. Each core computes its head end-to-end
(QKV projections, scores, softmax, PV, output projection), partials are
summed with an on-device AllReduce; host takes core 0's result.

Math restructuring (verified vs reference to fp32 precision):
  - scores scale 1/sqrt(D) folded into Wq/bq on host.
  - bk drops out of softmax entirely (constant per score row).
  - bv contributes heads += 1*bv (softmax rows sum to 1), so bv/bo become a
    single host-side constant row c = sum_h bv[h] @ Wo_h + bo added at the end.
  - No softmax max-subtraction needed: score std ~0.33, |scores| < ~2.5.

All matmuls run in float32r (FP22: full PE rate for N>=256, ~13-bit mantissa).
Activations stream transposed (feature dim on partitions) -- q/k/v are
pre-transposed on host so every matmul operand has its contraction dim on
partitions with zero on-device transposes.
"""
import os
import sys

sys.path.insert(0, "/opt/trn_rl_repo")
sys.path.insert(0, "/root/.axon_site")

import numpy as np

import concourse.bacc as bacc
import concourse.mybir as mybir
from concourse.tile import TileContext
from concourse import bass_utils

P = 128
B, S, D, H = 2, 2048, 512, 8
NCORES = 8
DT = D // P          # 4 feature tiles
MC = S // 512        # 4 m-chunks of 512 per batch
KT = S // P          # 16 km tiles per batch
F32 = mybir.dt.float32
F32R = mybir.dt.float32r

_NC_CACHE = {}


def _build_nc():
    nc = bacc.Bacc("TRN2", target_bir_lowering=False, debug=False,
                   num_devices=NCORES)

    qT = nc.dram_tensor("qT", [B, D, S], F32, kind="ExternalInput")
    kT = nc.dram_tensor("kT", [B, D, S], F32, kind="ExternalInput")
    vT = nc.dram_tensor("vT", [B, D, S], F32, kind="ExternalInput")
    wq = nc.dram_tensor("wq", [D, D], F32, kind="ExternalInput")
    wk = nc.dram_tensor("wk", [D, D], F32, kind="ExternalInput")
    wv = nc.dram_tensor("wv", [D, D], F32, kind="ExternalInput")
    wo = nc.dram_tensor("wo", [D, D], F32, kind="ExternalInput")
    bqv = nc.dram_tensor("bqv", [D], F32, kind="ExternalInput")
    ones128 = nc.dram_tensor("ones128", [P, P], F32, kind="ExternalInput")
    onesinv = nc.dram_tensor("onesinv", [P, 2], F32, kind="ExternalInput")
    out = nc.dram_tensor("out", [B, S, D], F32, kind="ExternalOutput")

    ar_out = [
        nc.dram_tensor(f"ar_out{b}", [S, D], F32, addr_space="Shared")
        for b in range(B)
    ]

    with TileContext(nc) as tc:
        with (
            tc.tile_pool(name="consts", bufs=1) as consts,
            tc.tile_pool(name="xstream", bufs=3) as xstream,
            tc.tile_pool(name="qts", bufs=2) as qts,
            tc.tile_pool(name="big", bufs=1) as big,
            tc.tile_pool(name="small", bufs=3) as small,
            tc.tile_pool(name="ostage", bufs=3) as ostage,
            tc.tile_pool(name="rot", bufs=3, space="PSUM") as rot,
            tc.tile_pool(name="psout", bufs=1, space="PSUM") as psout,
            tc.tile_pool(name="psden", bufs=1, space="PSUM") as psden,
            tc.tile_pool(name="dram", bufs=1, space="DRAM") as dram,
        ):
            # ---- constants ----
            def load_w(t):
                w_sb = consts.tile([P, DT, D], F32R, name=t.name + "_sb")
                nc.sync.dma_start(
                    w_sb[:],
                    t[:].rearrange("(dt p) e -> p dt e", p=P).bitcast(F32R),
                )
                return w_sb

            wq_sb, wk_sb, wv_sb, wo_sb = (load_w(t) for t in (wq, wk, wv, wo))
            bq_sb = consts.tile([P, DT], F32, name="bq_sb")
            nc.sync.dma_start(bq_sb[:], bqv[:].rearrange("(t p) -> p t", p=P))
            ones_sb = consts.tile([P, P], F32R, name="ones_sb")
            nc.sync.dma_start(ones_sb[:], ones128[:].bitcast(F32R))
            oinv_sb = consts.tile([P, 2], F32R, name="oinv_sb")
            nc.sync.dma_start(oinv_sb[:], onesinv[:].bitcast(F32R))

            partial = [dram.tile([S, D], F32, name=f"partial{b}") for b in range(B)]

            for b in range(B):
                # ---- K/V preparation: KT_b [e, km], V_b [km, e] ----
                KT_sb = big.tile([P, DT, S], F32R, tag="KT")
                V_sb = big.tile([P, KT, D], F32R, tag="V")
                for mc in range(MC):
                    ksl = slice(mc * 512, (mc + 1) * 512)
                    kT_ch = xstream.tile([P, DT, 512], F32R, tag="xT")
                    nc.sync.dma_start(
                        kT_ch[:],
                        kT[b].rearrange("(dt p) s -> p dt s", p=P)[:, :, ksl]
                        .bitcast(F32R),
                    )
                    for et in range(DT):
                        ps = rot.tile([P, 512], F32, tag="ps")
                        for dt in range(DT):
                            nc.tensor.matmul(
                                ps[:],
                                lhsT=wk_sb[:, dt, et * P:(et + 1) * P],
                                rhs=kT_ch[:, dt, :],
                                start=(dt == 0), stop=(dt == DT - 1),
                            )
                        nc.vector.tensor_copy(KT_sb[:, et, ksl], ps[:])
                    vT_ch = xstream.tile([P, DT, 512], F32R, tag="xT")
                    nc.sync.dma_start(
                        vT_ch[:],
                        vT[b].rearrange("(dt p) s -> p dt s", p=P)[:, :, ksl]
                        .bitcast(F32R),
                    )
                    for kt4 in range(4):
                        kt = mc * 4 + kt4
                        ps = rot.tile([P, 512], F32, tag="ps")
                        for dt in range(DT):
                            nc.tensor.matmul(
                                ps[:],
                                lhsT=vT_ch[:, dt, kt4 * P:(kt4 + 1) * P],
                                rhs=wv_sb[:, dt, :],
                                start=(dt == 0), stop=(dt == DT - 1),
                            )
                        nc.vector.tensor_copy(V_sb[:, kt, :], ps[:])

                # ---- attention, one 512-wide query chunk at a time ----
                for qc in range(MC):
                    qsl = slice(qc * 512, (qc + 1) * 512)
                    # QT chunk: project q on demand
                    qT_ch = xstream.tile([P, DT, 512], F32R, tag="xT")
                    nc.sync.dma_start(
                        qT_ch[:],
                        qT[b].rearrange("(dt p) s -> p dt s", p=P)[:, :, qsl]
                        .bitcast(F32R),
                    )
                    QTc = qts.tile([P, DT, 512], F32R, tag="QT")
                    for et in range(DT):
                        ps = rot.tile([P, 512], F32, tag="ps")
                        for dt in range(DT):
                            nc.tensor.matmul(
                                ps[:],
                                lhsT=wq_sb[:, dt, et * P:(et + 1) * P],
                                rhs=qT_ch[:, dt, :],
                                start=(dt == 0), stop=(dt == DT - 1),
                            )
                        nc.vector.tensor_scalar_add(
                            QTc[:, et, :], ps[:], bq_sb[:, et:et + 1]
                        )
                    # scoresT + exp -> PT [km, qm]
                    PT = big.tile([P, KT, 512], F32R, tag="PT")
                    for kt in range(KT):
                        ps = rot.tile([P, 512], F32, tag="ps")
                        for et in range(DT):
                            nc.tensor.matmul(
                                ps[:],
                                lhsT=KT_sb[:, et, kt * P:(kt + 1) * P],
                                rhs=QTc[:, et, :],
                                start=(et == 0), stop=(et == DT - 1),
                            )
                        nc.scalar.activation(
                            PT[:, kt, :], ps[:],
                            mybir.ActivationFunctionType.Exp,
                        )
                    # PV (transposed output) + denominator row
                    outT_ps = psout.tile([P, DT, 512], F32, tag="outT")
                    denB_ps = psden.tile([P, 512], F32, tag="denB")
                    for kt in range(KT):
                        for et in range(DT):
                            nc.tensor.matmul(
                                outT_ps[:, et, :],
                                lhsT=V_sb[:, kt, et * P:(et + 1) * P],
                                rhs=PT[:, kt, :],
                                start=(kt == 0), stop=(kt == KT - 1),
                            )
                        nc.tensor.matmul(
                            denB_ps[:],
                            lhsT=ones_sb[:],
                            rhs=PT[:, kt, :],
                            start=(kt == 0), stop=(kt == KT - 1),
                        )
                    # denom -> per-partition reciprocal [qm-tile, 1]
                    denB_sb = small.tile([P, 512], F32R, tag="denB_sb")
                    nc.vector.tensor_copy(denB_sb[:], denB_ps[:])
                    denT_ps = rot.tile([P, 512], F32, tag="ps")
                    for t in range(4):
                        nc.tensor.matmul(
                            denT_ps[:, 2 * t:2 * t + 2],
                            lhsT=denB_sb[:, t * P:(t + 1) * P],
                            rhs=oinv_sb[:],
                            start=True, stop=True,
                        )
                    recipT = small.tile([P, 8], F32, tag="recipT")
                    nc.vector.reciprocal(recipT[:], denT_ps[:, 0:8])
                    # out-projection: partial[qm, do] = (OT^T @ Wo) * recip
                    OT_sb = small.tile([P, DT, 512], F32R, tag="OT")
                    for et in range(DT):
                        nc.vector.tensor_copy(OT_sb[:, et, :], outT_ps[:, et, :])
                    for t in range(4):
                        ps = rot.tile([P, 512], F32, tag="ps")
                        for et in range(DT):
                            nc.tensor.matmul(
                                ps[:],
                                lhsT=OT_sb[:, et, t * P:(t + 1) * P],
                                rhs=wo_sb[:, et, :],
                                start=(et == 0), stop=(et == DT - 1),
                            )
                        o_sb = ostage.tile([P, 512], F32, tag="o")
                        nc.vector.tensor_scalar_mul(
                            o_sb[:], ps[:], recipT[:, 2 * t:2 * t + 1]
                        )
                        row = qc * 512 + t * P
                        nc.sync.dma_start(partial[b][row:row + P, :], o_sb[:])

                # per-batch AllReduce (overlaps with next batch's compute)
                nc.gpsimd.collective_compute(
                    "AllReduce",
                    mybir.AluOpType.add,
                    replica_groups=[list(range(NCORES))],
                    ins=[partial[b][:].opt()],
                    outs=[ar_out[b][:].opt()],
                )
                nc.gpsimd.dma_start(out[b], ar_out[b][:])

    nc.compile()
    return nc


def kernel(q, k, v, Wq, Wk, Wv, bq, bk, bv, Wo, bo):
    if "nc" not in _NC_CACHE:
        _NC_CACHE["nc"] = _build_nc()
    nc = _NC_CACHE["nc"]

    q = np.asarray(q, dtype=np.float32)
    k = np.asarray(k, dtype=np.float32)
    v = np.asarray(v, dtype=np.float32)
    Wq = np.asarray(Wq, dtype=np.float32)
    Wk = np.asarray(Wk, dtype=np.float32)
    Wv = np.asarray(Wv, dtype=np.float32)
    bq = np.asarray(bq, dtype=np.float32)
    bv = np.asarray(bv, dtype=np.float32)
    Wo = np.asarray(Wo, dtype=np.float32)
    bo = np.asarray(bo, dtype=np.float32)

    scale = np.float32(1.0 / np.sqrt(D))
    qT = np.ascontiguousarray(q.transpose(0, 2, 1))
    kT = np.ascontiguousarray(k.transpose(0, 2, 1))
    vT = np.ascontiguousarray(v.transpose(0, 2, 1))
    ones128 = np.ones((P, P), dtype=np.float32)
    onesinv = np.full((P, 2), 1.0 / P, dtype=np.float32)

    in_maps = []
    for h in range(NCORES):
        in_maps.append({
            "qT": qT, "kT": kT, "vT": vT,
            "wq": np.ascontiguousarray(Wq[h] * scale),
            "wk": np.ascontiguousarray(Wk[h]),
            "wv": np.ascontiguousarray(Wv[h]),
            "wo": np.ascontiguousarray(Wo[h * D:(h + 1) * D, :]),
            "bqv": np.ascontiguousarray(bq[h] * scale),
            "ones128": ones128,
            "onesinv": onesinv,
        })

    trace = bool(int(os.environ.get("KERNEL_TRACE", "0")))
    if trace:
        try:
            import trace_hook
            trace_hook.install()
        except Exception:
            pass
    res = bass_utils.run_bass_kernel_spmd(
        nc, in_maps, core_ids=list(range(NCORES)), trace=trace
    )
    _NC_CACHE["last_result"] = res

    out = np.array(res.results[0]["out"])  # [B, S, D]
    c_const = sum(bv[h] @ Wo[h * D:(h + 1) * D, :] for h in range(H)) + bo
    out += c_const[None, None, :].astype(np.float32)
    return out.astype(np.float32)


# revision 4
# speedup vs baseline: 1.1041x; 1.1041x over previous
"""Head-parallel MultiHeadAttention kernel for 8 Trainium2 NeuronCores.

Problem: B=2, S=2048, D=512, H=8, per-head full-width projections.
Sharding: head h -> core h. Each core computes its head end-to-end
(QKV projections, scores, softmax, PV, output projection), partials are
summed with an on-device AllReduce; host takes core 0's result.

Math restructuring (verified vs reference to fp32 precision):
  - scores scale 1/sqrt(D) folded into Wq/bq on host.
  - bk drops out of softmax entirely (constant per score row).
  - bv contributes heads += 1*bv (softmax rows sum to 1), so bv/bo become a
    single host-side constant row c = sum_h bv[h] @ Wo_h + bo added at the end.
  - No softmax max-subtraction needed: score std ~0.33, |scores| < ~2.5.

All matmuls run in float32r (FP22: full PE rate for N>=256, ~13-bit mantissa).
Activations stream transposed (feature dim on partitions) -- q/k/v are
pre-transposed on host so every matmul operand has its contraction dim on
partitions with zero on-device transposes.
"""
import os
import sys

sys.path.insert(0, "/opt/trn_rl_repo")
sys.path.insert(0, "/root/.axon_site")

import numpy as np

import concourse.bacc as bacc
import concourse.mybir as mybir
from concourse.tile import TileContext
from concourse import bass_utils

P = 128
B, S, D, H = 2, 2048, 512, 8
NCORES = 8
DT = D // P          # 4 feature tiles
MC = S // 512        # 4 m-chunks of 512 per batch
KT = S // P          # 16 km tiles per batch
F32 = mybir.dt.float32
F32R = mybir.dt.float32r

_NC_CACHE = {}


def _build_nc():
    nc = bacc.Bacc("TRN2", target_bir_lowering=False, debug=False,
                   num_devices=NCORES)

    qT = nc.dram_tensor("qT", [B, D, S], F32, kind="ExternalInput")
    kT = nc.dram_tensor("kT", [B, D, S], F32, kind="ExternalInput")
    vT = nc.dram_tensor("vT", [B, D, S], F32, kind="ExternalInput")
    wq = nc.dram_tensor("wq", [D, D], F32, kind="ExternalInput")
    wk = nc.dram_tensor("wk", [D, D], F32, kind="ExternalInput")
    wv = nc.dram_tensor("wv", [D, D], F32, kind="ExternalInput")
    wo = nc.dram_tensor("wo", [D, D], F32, kind="ExternalInput")
    bqv = nc.dram_tensor("bqv", [D], F32, kind="ExternalInput")
    ones128 = nc.dram_tensor("ones128", [P, P], F32, kind="ExternalInput")
    onesinv = nc.dram_tensor("onesinv", [P, 2], F32, kind="ExternalInput")
    out = nc.dram_tensor("out", [B, S, D], F32, kind="ExternalOutput")

    ar_out = [
        nc.dram_tensor(f"ar_out{b}_{qc}", [512, D], F32, addr_space="Shared")
        for b in range(B) for qc in range(MC)
    ]

    with TileContext(nc) as tc:
        with (
            tc.tile_pool(name="consts", bufs=1) as consts,
            tc.tile_pool(name="xstream", bufs=3) as xstream,
            tc.tile_pool(name="qts", bufs=2) as qts,
            tc.tile_pool(name="big", bufs=1) as big,
            tc.tile_pool(name="small", bufs=3) as small,
            tc.tile_pool(name="ostage", bufs=3) as ostage,
            tc.tile_pool(name="rot", bufs=3, space="PSUM") as rot,
            tc.tile_pool(name="psout", bufs=1, space="PSUM") as psout,
            tc.tile_pool(name="psden", bufs=1, space="PSUM") as psden,
            tc.tile_pool(name="dram", bufs=1, space="DRAM") as dram,
        ):
            # ---- constants ----
            def load_w(t):
                w_sb = consts.tile([P, DT, D], F32R, name=t.name + "_sb")
                nc.sync.dma_start(
                    w_sb[:],
                    t[:].rearrange("(dt p) e -> p dt e", p=P).bitcast(F32R),
                )
                return w_sb

            wk_sb, wv_sb, wq_sb, wo_sb = (load_w(t) for t in (wk, wv, wq, wo))
            bq_sb = consts.tile([P, DT], F32, name="bq_sb")
            nc.sync.dma_start(bq_sb[:], bqv[:].rearrange("(t p) -> p t", p=P))
            ones_sb = consts.tile([P, P], F32R, name="ones_sb")
            nc.sync.dma_start(ones_sb[:], ones128[:].bitcast(F32R))
            oinv_sb = consts.tile([P, 2], F32R, name="oinv_sb")
            nc.sync.dma_start(oinv_sb[:], onesinv[:].bitcast(F32R))

            partial = [
                dram.tile([512, D], F32, name=f"partial{b}_{qc}")
                for b in range(B) for qc in range(MC)
            ]

            for b in range(B):
                # ---- K/V preparation: KT_b [e, km], V_b [km, e] ----
                KT_sb = big.tile([P, DT, S], F32R, tag="KT")
                V_sb = big.tile([P, KT, D], F32R, tag="V")
                for mc in range(MC):
                    ksl = slice(mc * 512, (mc + 1) * 512)
                    kT_ch = xstream.tile([P, DT, 512], F32R, tag="xT")
                    nc.sync.dma_start(
                        kT_ch[:],
                        kT[b].rearrange("(dt p) s -> p dt s", p=P)[:, :, ksl]
                        .bitcast(F32R),
                    )
                    for et in range(DT):
                        ps = rot.tile([P, 512], F32, tag="ps")
                        for dt in range(DT):
                            nc.tensor.matmul(
                                ps[:],
                                lhsT=wk_sb[:, dt, et * P:(et + 1) * P],
                                rhs=kT_ch[:, dt, :],
                                start=(dt == 0), stop=(dt == DT - 1),
                            )
                        nc.vector.tensor_copy(KT_sb[:, et, ksl], ps[:])
                    vT_ch = xstream.tile([P, DT, 512], F32R, tag="xT")
                    nc.sync.dma_start(
                        vT_ch[:],
                        vT[b].rearrange("(dt p) s -> p dt s", p=P)[:, :, ksl]
                        .bitcast(F32R),
                    )
                    for kt4 in range(4):
                        kt = mc * 4 + kt4
                        ps = rot.tile([P, 512], F32, tag="ps")
                        for dt in range(DT):
                            nc.tensor.matmul(
                                ps[:],
                                lhsT=vT_ch[:, dt, kt4 * P:(kt4 + 1) * P],
                                rhs=wv_sb[:, dt, :],
                                start=(dt == 0), stop=(dt == DT - 1),
                            )
                        nc.vector.tensor_copy(V_sb[:, kt, :], ps[:])

                # ---- attention, one 512-wide query chunk at a time ----
                for qc in range(MC):
                    qsl = slice(qc * 512, (qc + 1) * 512)
                    # QT chunk: project q on demand
                    qT_ch = xstream.tile([P, DT, 512], F32R, tag="xT")
                    nc.sync.dma_start(
                        qT_ch[:],
                        qT[b].rearrange("(dt p) s -> p dt s", p=P)[:, :, qsl]
                        .bitcast(F32R),
                    )
                    QTc = qts.tile([P, DT, 512], F32R, tag="QT")
                    for et in range(DT):
                        ps = rot.tile([P, 512], F32, tag="ps")
                        for dt in range(DT):
                            nc.tensor.matmul(
                                ps[:],
                                lhsT=wq_sb[:, dt, et * P:(et + 1) * P],
                                rhs=qT_ch[:, dt, :],
                                start=(dt == 0), stop=(dt == DT - 1),
                            )
                        nc.vector.tensor_scalar_add(
                            QTc[:, et, :], ps[:], bq_sb[:, et:et + 1]
                        )
                    # scoresT + exp -> PT [km, qm]
                    PT = big.tile([P, KT, 512], F32R, tag="PT")
                    for kt in range(KT):
                        ps = rot.tile([P, 512], F32, tag="ps")
                        for et in range(DT):
                            nc.tensor.matmul(
                                ps[:],
                                lhsT=KT_sb[:, et, kt * P:(kt + 1) * P],
                                rhs=QTc[:, et, :],
                                start=(et == 0), stop=(et == DT - 1),
                            )
                        nc.scalar.activation(
                            PT[:, kt, :], ps[:],
                            mybir.ActivationFunctionType.Exp,
                        )
                    # PV (transposed output) + denominator row
                    outT_ps = psout.tile([P, DT, 512], F32, tag="outT")
                    denB_ps = psden.tile([P, 512], F32, tag="denB")
                    for kt in range(KT):
                        for et in range(DT):
                            nc.tensor.matmul(
                                outT_ps[:, et, :],
                                lhsT=V_sb[:, kt, et * P:(et + 1) * P],
                                rhs=PT[:, kt, :],
                                start=(kt == 0), stop=(kt == KT - 1),
                            )
                        nc.tensor.matmul(
                            denB_ps[:],
                            lhsT=ones_sb[:],
                            rhs=PT[:, kt, :],
                            start=(kt == 0), stop=(kt == KT - 1),
                        )
                    # denom -> per-partition reciprocal [qm-tile, 1]
                    denB_sb = small.tile([P, 512], F32R, tag="denB_sb")
                    nc.vector.tensor_copy(denB_sb[:], denB_ps[:])
                    denT_ps = rot.tile([P, 512], F32, tag="ps")
                    for t in range(4):
                        nc.tensor.matmul(
                            denT_ps[:, 2 * t:2 * t + 2],
                            lhsT=denB_sb[:, t * P:(t + 1) * P],
                            rhs=oinv_sb[:],
                            start=True, stop=True,
                        )
                    recipT = small.tile([P, 8], F32, tag="recipT")
                    nc.vector.reciprocal(recipT[:], denT_ps[:, 0:8])
                    # out-projection: partial[qm, do] = (OT^T @ Wo) * recip
                    OT_sb = small.tile([P, DT, 512], F32R, tag="OT")
                    for et in range(DT):
                        nc.vector.tensor_copy(OT_sb[:, et, :], outT_ps[:, et, :])
                    for t in range(4):
                        ps = rot.tile([P, 512], F32, tag="ps")
                        for et in range(DT):
                            nc.tensor.matmul(
                                ps[:],
                                lhsT=OT_sb[:, et, t * P:(t + 1) * P],
                                rhs=wo_sb[:, et, :],
                                start=(et == 0), stop=(et == DT - 1),
                            )
                        o_sb = ostage.tile([P, 512], F32, tag="o")
                        nc.vector.tensor_scalar_mul(
                            o_sb[:], ps[:], recipT[:, 2 * t:2 * t + 1]
                        )
                        row = t * P
                        pidx = b * MC + qc
                        nc.sync.dma_start(partial[pidx][row:row + P, :], o_sb[:])

                    # per-chunk AllReduce: overlaps remaining compute
                    pidx = b * MC + qc
                    nc.gpsimd.collective_compute(
                        "AllReduce",
                        mybir.AluOpType.add,
                        replica_groups=[list(range(NCORES))],
                        ins=[partial[pidx][:].opt()],
                        outs=[ar_out[pidx][:].opt()],
                    )
                    nc.gpsimd.dma_start(
                        out[b, qc * 512:(qc + 1) * 512, :], ar_out[pidx][:]
                    )

    nc.compile()
    return nc


def kernel(q, k, v, Wq, Wk, Wv, bq, bk, bv, Wo, bo):
    if "nc" not in _NC_CACHE:
        _NC_CACHE["nc"] = _build_nc()
    nc = _NC_CACHE["nc"]

    q = np.asarray(q, dtype=np.float32)
    k = np.asarray(k, dtype=np.float32)
    v = np.asarray(v, dtype=np.float32)
    Wq = np.asarray(Wq, dtype=np.float32)
    Wk = np.asarray(Wk, dtype=np.float32)
    Wv = np.asarray(Wv, dtype=np.float32)
    bq = np.asarray(bq, dtype=np.float32)
    bv = np.asarray(bv, dtype=np.float32)
    Wo = np.asarray(Wo, dtype=np.float32)
    bo = np.asarray(bo, dtype=np.float32)

    scale = np.float32(1.0 / np.sqrt(D))
    qT = np.ascontiguousarray(q.transpose(0, 2, 1))
    kT = np.ascontiguousarray(k.transpose(0, 2, 1))
    vT = np.ascontiguousarray(v.transpose(0, 2, 1))
    ones128 = np.ones((P, P), dtype=np.float32)
    onesinv = np.full((P, 2), 1.0 / P, dtype=np.float32)

    in_maps = []
    for h in range(NCORES):
        in_maps.append({
            "qT": qT, "kT": kT, "vT": vT,
            "wq": np.ascontiguousarray(Wq[h] * scale),
            "wk": np.ascontiguousarray(Wk[h]),
            "wv": np.ascontiguousarray(Wv[h]),
            "wo": np.ascontiguousarray(Wo[h * D:(h + 1) * D, :]),
            "bqv": np.ascontiguousarray(bq[h] * scale),
            "ones128": ones128,
            "onesinv": onesinv,
        })

    trace = bool(int(os.environ.get("KERNEL_TRACE", "0")))
    if trace:
        try:
            import trace_hook
            trace_hook.install()
        except Exception:
            pass
    res = bass_utils.run_bass_kernel_spmd(
        nc, in_maps, core_ids=list(range(NCORES)), trace=trace
    )
    _NC_CACHE["last_result"] = res

    out = np.array(res.results[0]["out"])  # [B, S, D]
    c_const = sum(bv[h] @ Wo[h * D:(h + 1) * D, :] for h in range(H)) + bo
    out += c_const[None, None, :].astype(np.float32)
    return out.astype(np.float32)
